# revision 1
# baseline (speedup 1.0000x reference)
"""CAM (channel-attention) module kernel for Trainium2.

Computes, per batch b:
    q      = x[b].reshape(C, H*W)
    E      = q @ q.T                                  # [C, C] channel Gram matrix
    A[i,j] = softmax_j(rowmax_i(E) - E[i,j])          # suppression softmax
           = exp(rowmin_i(E) - E[i,j]) / Z_i
    out[b] = gamma * (A @ q) + x[b]

Distribution: pure data-parallel over batch B=16 across 8 NeuronCores
(2 batches per core); gamma replicated. No collectives.

Per-core kernel strategy (all matmuls on PE in float32r, 1 cyc/row when the
moving free dim >= 256):
  1. q loaded natural-layout [128, 4, 4096] (partition = channel % 128),
     exact fp32 (the residual path needs the original bits).
  2. qT built on-chip via PE transpose-mode in 128-column chunks,
     software-pipelined with the Gram matmul; the PSUM->SBUF copy is a
     rounding cast to float32r (walrus requires f32r matmul operands to
     come from a rounding producer; fp32 matmul would be 4 cyc/row).
  3. E computed block-upper-triangular only (symmetry); the strictly-lower
     128x128 blocks are mirrored with exact fp32 PE transposes (E feeds
     exp directly, so f32r rounding there would be a real error).
  4. S = exp(rowmin - E) fused on ScalarE (bias=rowmin, scale=-1) with
     accum_out producing Z = sum_j S in the same instruction; written as
     float32r so S is a legal transpose/matmul operand.
  5. S transposed 128x128-blockwise on PE -> ST (attention^T, stationary
     operand of the second matmul).
  6. U = ST.T @ qr on PE (qr = f32r cast of a q column chunk, cast 2
     chunks ahead on ScalarE/VectorE); epilogue out = (gamma/Z)*U + x is
     a single VectorE scalar_tensor_tensor reading the exact fp32 q.
  7. Cross-batch software pipelining: batch b's transpose+Gram chunks are
     burst-interleaved (4 chunks : 1 s-group) with batch b-1's
     attention-apply, so the PE never idles long enough for the HAM clock
     gate to re-throttle it to 1.2 GHz.
"""

import sys

import numpy as np

if "/opt/trn_rl_repo" not in sys.path:
    sys.path.insert(0, "/opt/trn_rl_repo")

B, C, H, W = 16, 512, 64, 64
N = H * W                # 4096 spatial positions
P = 128                  # partitions
CT = C // P              # 4 channel tiles
KT = N // P              # 32 contraction chunks for the Gram matmul
FD = 512                 # matmul moving free dim / PSUM bank width (fp32)
NCH = N // FD            # 8 output column chunks
N_CORES = 8
BPC = B // N_CORES       # 2 batches per core

# Moving-operand start column for the upper-triangular Gram matmul. Row-tile 3
# widens from 128 to 256 columns: float32r only streams at 1 cyc/row when the
# output free dim is >= 256, so recomputing block (3,2) is cheaper than a
# 128-wide f32r matmul.
MVSTART = [0, 128, 256, 256]

_CACHE = {}


def _build_nc():
    from contextlib import ExitStack

    import concourse.bacc as bacc
    import concourse.tile as tile
    from concourse import mybir
    from concourse.masks import make_identity

    f32 = mybir.dt.float32
    f32r = mybir.dt.float32r
    AF = mybir.ActivationFunctionType
    ALU = mybir.AluOpType

    nc = bacc.Bacc(None, target_bir_lowering=False)
    # x stays float32 end-to-end on the load path: the DMA cast unit ROUNDS
    # when the destination dtype is float32r (measured: 11-bit mantissa),
    # which would corrupt the residual. float32r operands for the PE are
    # instead produced by engine cast-copies (ScalarE/VectorE).
    x_d = nc.dram_tensor("x", [BPC, C, N], f32, kind="ExternalInput")
    g_d = nc.dram_tensor("gamma", [1], f32, kind="ExternalInput")
    o_d = nc.dram_tensor("out", [BPC, C, N], f32, kind="ExternalOutput")

    with ExitStack() as ctx:
        tc = ctx.enter_context(tile.TileContext(nc))
        singles = ctx.enter_context(tc.tile_pool(name="singles", bufs=1))
        bigq = ctx.enter_context(tc.tile_pool(name="bigq", bufs=2))
        qtp = ctx.enter_context(tc.tile_pool(name="qtp", bufs=5))
        qrp = ctx.enter_context(tc.tile_pool(name="qrp", bufs=3))
        mats = ctx.enter_context(tc.tile_pool(name="mats", bufs=4))
        outp = ctx.enter_context(tc.tile_pool(name="outp", bufs=3))
        smallp = ctx.enter_context(tc.tile_pool(name="small", bufs=8))
        psp = ctx.enter_context(tc.tile_pool(name="ps", bufs=8, space="PSUM"))

        def ps_tile():
            return psp.tile([P, FD], f32, tag="ps", name="ps")

        LOOK = 2

        def emit_load(b):
            xb = x_d[b].rearrange("(ct p) n -> p ct n", p=P)
            ob = o_d[b].rearrange("(ct p) n -> p ct n", p=P)
            q = bigq.tile([P, CT, N], f32, tag="q")
            for s in range(NCH):
                nc.sync.dma_start(
                    out=q[:, :, s * FD : (s + 1) * FD],
                    in_=xb[:, :, s * FD : (s + 1) * FD],
                )
            return {"q": q, "xb": xb, "ob": ob}

        def emit_tr(st, k):
            q = st["q"]
            pst = psp.tile([P, FD], f32, tag="ps", name="pstr")
            for t in range(CT):
                nc.tensor.transpose(
                    pst[:, t * P : (t + 1) * P],
                    q[:, t, k * P : (k + 1) * P],
                    ident[:],
                )
            # rounding cast f32 -> f32r makes qk a legal f32r operand
            qk = qtp.tile([P, C], f32r, tag="qt")
            if k % 4 == 3:
                nc.vector.tensor_copy(qk[:], pst[:])
            else:
                nc.scalar.copy(qk[:], pst[:])
            st["qt"][k] = qk

        def emit_mm1(st, k):
            qkr = st["qt"][k]
            psE = st["psE"]
            for t in range(CT):
                w = C - MVSTART[t]
                nc.tensor.matmul(
                    psE[t][:, :w],
                    qkr[:, t * P : (t + 1) * P],
                    qkr[:, MVSTART[t] :],
                    start=(k == 0),
                    stop=(k == KT - 1),
                )

        def emit_cast(st, s):
            q = st["q"]
            qr = qrp.tile([P, CT, FD], f32r, tag="qr")
            if s % 4 == 3:
                nc.vector.tensor_copy(qr[:], q[:, :, s * FD : (s + 1) * FD])
            else:
                nc.scalar.copy(qr[:], q[:, :, s * FD : (s + 1) * FD])
            st["qrs"][s] = qr

        def emit_mm2_s(st, s, split_epi=False):
            # one s-chunk of mm2 + epilogue: 4 psU groups
            if s == 0:
                emit_cast(st, 0)
                emit_cast(st, 1)
            if s + 2 < NCH:
                emit_cast(st, s + 2)
            qr = st["qrs"][s]
            q, ob, ST, grz = st["q"], st["ob"], st["ST"], st["grz"]
            for t in range(CT):
                pu = ps_tile()
                for jt in range(CT):
                    nc.tensor.matmul(
                        pu[:],
                        ST[jt][:, t * P : (t + 1) * P],
                        qr[:, jt, :],
                        start=(jt == 0),
                        stop=(jt == CT - 1),
                    )
                ot = outp.tile([P, FD], f32, tag="ot")
                if split_epi and t < 2:
                    # kernel tail: spread the epilogue across ScalarE+VectorE
                    # (t<2 split, t>=2 fused) so neither engine outpaces the
                    # PE and the post-matmul drain stays short
                    nc.scalar.mul(ot[:], pu[:], grz[t][:])
                    nc.vector.tensor_add(
                        ot[:], ot[:], q[:, t, s * FD : (s + 1) * FD]
                    )
                else:
                    # out = (U * gamma/Z) + x in one VectorE op
                    nc.vector.scalar_tensor_tensor(
                        ot[:],
                        pu[:],
                        grz[t][:],
                        q[:, t, s * FD : (s + 1) * FD],
                        op0=ALU.mult,
                        op1=ALU.add,
                    )
                nc.sync.dma_start(out=ob[:, t, s * FD : (s + 1) * FD], in_=ot[:])

        def emit_gram(st, prev, skip_chunks=0):
            """Transposes + Gram matmul for `st`, burst-interleaved with the
            previous batch's attention-apply (mm2) so PE never idles long
            enough for the HAM clock gate to re-throttle."""
            st["psE"] = [ps_tile() for _ in range(CT)]
            if "qt" not in st:
                st["qt"] = [None] * KT
            for k in range(skip_chunks, KT):
                emit_tr(st, k)
                if k >= LOOK:
                    emit_mm1(st, k - LOOK)
                # only 6 of 8 s-groups here: the last two fill this batch's
                # own softmax phase, where the PE would otherwise idle
                if prev is not None and k % 4 == 3 and k // 4 < NCH - 2:
                    emit_mm2_s(prev, k // 4)
            for k in range(KT - LOOK, KT):
                emit_mm1(st, k)

        def emit_softmax(st, prev=None):
            # ---- copy E out of PSUM; mirror strictly-lower blocks ----
            psE = st["psE"]
            E = []
            for t in range(CT):
                e = mats.tile([P, FD], f32, tag="E")
                w = C - MVSTART[t]
                if t % 2 == 0:
                    nc.scalar.copy(e[:, MVSTART[t] :], psE[t][:, :w])
                else:
                    nc.vector.tensor_copy(e[:, MVSTART[t] :], psE[t][:, :w])
                E.append(e)
            # E[t][:, s-block] = E[s][:, t-block].T for s < t (exact fp32
            # transposes: E magnitudes are ~4e3 and feed exp directly, so
            # float32r rounding here would be a real error).
            for t in range(1, CT):
                for s in range(t):
                    if t == 3 and s == 2:
                        continue  # computed directly via the widened row-tile 3
                    pm = ps_tile()
                    nc.tensor.transpose(
                        pm[:, :P], E[s][:, t * P : (t + 1) * P], ident[:]
                    )
                    if (t + s) % 2 == 0:
                        nc.scalar.copy(E[t][:, s * P : (s + 1) * P], pm[:, :P])
                    else:
                        nc.vector.tensor_copy(
                            E[t][:, s * P : (s + 1) * P], pm[:, :P]
                        )

            # deferred mm2 s-group of the previous batch keeps the PE busy
            # while the rowmin/exp chains run on VectorE/ScalarE; for the
            # first batch, the NEXT batch's first transposes fill in instead
            if prev is not None:
                emit_mm2_s(prev, NCH - 2)
            elif st.get("next") is not None:
                emit_tr(st["next"], 0)

            # ---- suppression softmax: S = exp(rowmin - E), Z = rowsum(S) ----
            S = []
            grz = []
            for t in range(CT):
                rm = smallp.tile([P, 1], f32, tag="rm")
                nc.vector.tensor_reduce(
                    rm[:], E[t][:], axis=mybir.AxisListType.X, op=ALU.min
                )
                s_t = mats.tile([P, FD], f32r, tag="S")
                z = smallp.tile([P, 1], f32, tag="z")
                nc.scalar.activation(
                    s_t[:], E[t][:], AF.Exp, bias=rm[:], scale=-1.0, accum_out=z[:]
                )
                S.append(s_t)
                rz = smallp.tile([P, 1], f32, tag="rz")
                nc.vector.reciprocal(rz[:], z[:])
                g = smallp.tile([P, 1], f32, tag="grz")
                nc.vector.tensor_mul(g[:], rz[:], gam[:])
                grz.append(g)

            if prev is not None:
                emit_mm2_s(prev, NCH - 1)
            elif st.get("next") is not None:
                emit_tr(st["next"], 1)

            # ---- ST = S.T (attention^T), 128x128 blocks on PE ----
            # Ordered by source tile t so each ST transpose can start as soon
            # as S[t] exists; 4 PSUM banks stay open across the t loop.
            pstS = [
                psp.tile([P, FD], f32r, tag="ps", name="pstS") for _ in range(CT)
            ]
            for t in range(CT):
                for jt in range(CT):
                    nc.tensor.transpose(
                        pstS[jt][:, t * P : (t + 1) * P],
                        S[t][:, jt * P : (jt + 1) * P],
                        identr[:],
                    )
            ST = []
            for jt in range(CT):
                stj = mats.tile([P, FD], f32r, tag="ST")
                if jt % 2 == 0:
                    nc.scalar.copy(stj[:], pstS[jt][:])
                else:
                    nc.vector.tensor_copy(stj[:], pstS[jt][:])
                ST.append(stj)
            st["ST"] = ST
            st["grz"] = grz
            st["qrs"] = [None] * NCH

        # ---- pipelined driver: batch b's Gram phase overlaps batch b-1's
        # attention-apply phase on the PE ----
        ident = singles.tile([P, P], f32)
        make_identity(nc, ident)
        identr = singles.tile([P, P], f32r)
        nc.vector.tensor_copy(identr[:], ident[:])  # rounding cast producer

        # gamma broadcast to all partitions as a per-partition scalar
        gam = singles.tile([P, 1], f32)
        nc.gpsimd.dma_start(out=gam[:], in_=g_d[:].to_broadcast([P, 1]))

        st0 = emit_load(0)
        emit_gram(st0, None)
        st1 = emit_load(1)
        st1["qt"] = [None] * KT
        st0["next"] = st1
        emit_softmax(st0, None)
        emit_gram(st1, st0, skip_chunks=2)
        emit_softmax(st1, st0)
        for s in range(NCH):
            emit_mm2_s(st1, s, split_epi=(s >= NCH - 4))

    nc.compile()
    return nc


def _get_nc():
    if "nc" not in _CACHE:
        _CACHE["nc"] = _build_nc()
    return _CACHE["nc"]


def kernel(x: np.ndarray, gamma: np.ndarray) -> np.ndarray:
    from concourse.bass_utils import run_bass_kernel_spmd

    nc = _get_nc()
    x = np.ascontiguousarray(np.asarray(x, dtype=np.float32))
    gamma = np.ascontiguousarray(np.asarray(gamma, dtype=np.float32))
    xs = x.reshape(B, C, N)
    in_maps = [
        {
            "x": np.ascontiguousarray(xs[c * BPC : (c + 1) * BPC]),
            "gamma": gamma,
        }
        for c in range(N_CORES)
    ]
    res = run_bass_kernel_spmd(nc, in_maps, core_ids=list(range(N_CORES)))
    out = np.stack([res.results[c]["out"] for c in range(N_CORES)], axis=0)
    return out.reshape(B, C, H, W)



# revision 22
# speedup vs baseline: 1.1968x; 1.1968x over previous
"""CAM (channel-attention) module kernel for Trainium2.

Computes, per batch b:
    q      = x[b].reshape(C, H*W)
    E      = q @ q.T                                  # [C, C] channel Gram matrix
    A[i,j] = softmax_j(rowmax_i(E) - E[i,j])          # suppression softmax
           = exp(rowmin_i(E) - E[i,j]) / Z_i
    out[b] = gamma * (A @ q) + x[b]
Distribution: pure data-parallel over batch B=16 across 8 NeuronCores
(2 batches per core); gamma replicated. No collectives.

Per-core kernel strategy (PE wall time is stream-cycles + ~45ns/inst, so
everything aims at 1-cycle/row streams and fewer, longer instructions):
  1. All 16 load DMAs (both batches) queued up front on one ring; batch
     0's first chunk is split so the first transpose starts early.
  2. q natural-layout [128, 4, 4096] exact fp32 (residual needs the
     bits); per-chunk fp16 casts qh feed the PE transposes (fp16 streams
     1 cyc/row vs fp32's two-pass LOW_HIGH mode, and keeps the same
     11-bit mantissa as float32r so the suppression argmin stays right).
  3. qT built 2 chunks per PSUM bank (fp16 halves the footprint), one
     PSUM->SBUF copy per pair; Gram matmul in fp16 with the tight
     block-triangle [512,384,256,128] (no f32r >=256 width rule).
  4. E accumulated fp32 in PSUM; strictly-lower blocks mirrored with
     exact fp32 PE transposes (E feeds exp directly).
  5. S = exp(rowmin - E) on ScalarE (bias=rowmin, scale=-1) with
     accum_out Z; S fp16. Softmax is a per-tile pipeline with almost
     nothing else contending for ACT/DVE.
  6. S transposed blockwise on PE (fp16, 1 cyc/row), packed to one
     fp8_e4m3 stationary STdr [128, jt, i] for DoubleRow.
  7. attention-apply in fp8 DoubleRow perf mode: 2 k-tiles per
     instruction halves the instruction count vs fp16. Moving operand =
     fp8 cast of q: batch 0 casts ride a lookahead ring on ScalarE;
     batch 1's are pre-cast on VectorE during its Gram phase so the
     kernel tail never waits on a cast.
  8. Epilogue out = (gamma/Z)*U + x reads exact fp32 q. Injected groups
     put one residual add on PoolE (slow but idle there); tail groups
     stay on VectorE/ScalarE.
  9. Stores batched one [128, 4, 512] DMA per s-chunk.
 10. Cross-batch pipelining: 6 of batch b-1's 8 attention-apply groups
     interleave into batch b's Gram phase; group 6's matmuls fill batch
     b's softmax PE bubble with the epilogue deferred past the softmax
     chain; group 7 runs right after the fp8 stationary is packed.
"""

import sys

import numpy as np

if "/opt/trn_rl_repo" not in sys.path:
    sys.path.insert(0, "/opt/trn_rl_repo")

B, C, H, W = 16, 512, 64, 64
N = H * W                # 4096 spatial positions
P = 128                  # partitions
CT = C // P              # 4 channel tiles
KT = N // P              # 32 contraction chunks for the Gram matmul
FD = 512                 # matmul moving free dim / PSUM bank width (fp32)
NCH = N // FD            # 8 output column chunks
N_CORES = 8
BPC = B // N_CORES       # 2 batches per core

# Moving-operand start column for the upper-triangular Gram matmul (fp16
# streams 1 cyc/row at any width, so the triangle is exact).
MVSTART = [0, 128, 256, 384]

_CACHE = {}


def _build_nc():
    from contextlib import ExitStack

    import concourse.bacc as bacc
    import concourse.tile as tile
    from concourse import mybir
    from concourse.masks import make_identity

    f32 = mybir.dt.float32
    f16 = mybir.dt.float16
    f8e4 = mybir.dt.float8e4
    AF = mybir.ActivationFunctionType
    ALU = mybir.AluOpType
    DR = mybir.MatmulPerfMode.DoubleRow

    nc = bacc.Bacc(None, target_bir_lowering=False)
    # x stays float32 end-to-end on the load path (the DMA cast unit would
    # round); reduced-precision PE operands come from engine casts.
    x_d = nc.dram_tensor("x", [BPC, C, N], f32, kind="ExternalInput")
    g_d = nc.dram_tensor("gamma", [1], f32, kind="ExternalInput")
    o_d = nc.dram_tensor("out", [BPC, C, N], f32, kind="ExternalOutput")

    with ExitStack() as ctx:
        tc = ctx.enter_context(tile.TileContext(nc))
        singles = ctx.enter_context(tc.tile_pool(name="singles", bufs=1))
        bigq = ctx.enter_context(tc.tile_pool(name="bigq", bufs=2))
        qhp = ctx.enter_context(tc.tile_pool(name="qhp", bufs=3))
        qtp = ctx.enter_context(tc.tile_pool(name="qtp", bufs=3))
        qmp = ctx.enter_context(tc.tile_pool(name="qmp", bufs=2))
        mats = ctx.enter_context(tc.tile_pool(name="mats", bufs=4))
        outp = ctx.enter_context(tc.tile_pool(name="outp", bufs=2))
        smallp = ctx.enter_context(tc.tile_pool(name="small", bufs=8))
        psp = ctx.enter_context(tc.tile_pool(name="ps", bufs=8, space="PSUM"))

        def ps_tile(name="ps"):
            return psp.tile([P, FD], f32, tag="ps", name=name)

        def emit_load(b, split_first=False):
            xb = x_d[b].rearrange("(ct p) n -> p ct n", p=P)
            ob = o_d[b].rearrange("(ct p) n -> p ct n", p=P)
            q = bigq.tile([P, CT, N], f32, tag="q")
            for s in range(NCH):
                if split_first and s == 0:
                    h = FD // 2
                    nc.sync.dma_start(out=q[:, :, 0:h], in_=xb[:, :, 0:h])
                    nc.sync.dma_start(out=q[:, :, h:FD], in_=xb[:, :, h:FD])
                    continue
                nc.sync.dma_start(
                    out=q[:, :, s * FD : (s + 1) * FD],
                    in_=xb[:, :, s * FD : (s + 1) * FD],
                )
            return {"q": q, "xb": xb, "ob": ob}

        def emit_qh(st, c, engine="s", halves=False):
            # fp16 cast of q chunk c (feeds the PE transposes)
            q = st["q"]
            qh = qhp.tile([P, CT, FD], f16, tag="qh", name="qh")
            if halves:
                # first transposes only need the first half: don't wait for
                # the whole chunk to land
                h = FD // 2
                nc.scalar.copy(qh[:, :, 0:h], q[:, :, c * FD : c * FD + h])
                nc.scalar.copy(qh[:, :, h:FD], q[:, :, c * FD + h : (c + 1) * FD])
            elif engine == "v":
                nc.vector.tensor_copy(qh[:], q[:, :, c * FD : (c + 1) * FD])
            else:
                nc.scalar.copy(qh[:], q[:, :, c * FD : (c + 1) * FD])
            st["qh"][c] = qh

        def emit_tr_pair(st, kk):
            # transpose chunks 2kk and 2kk+1 into one fp16 PSUM bank;
            # a single PSUM->SBUF copy yields both Gram operand chunks
            pst = psp.tile([P, 2, FD], f16, tag="ps", name="pstr")
            for i in range(2):
                k = 2 * kk + i
                qh = st["qh"][k // 4]
                for t in range(CT):
                    nc.tensor.transpose(
                        pst[:, i, t * P : (t + 1) * P],
                        qh[:, t, (k % 4) * P : (k % 4 + 1) * P],
                        identh[:],
                    )
            qk = qtp.tile([P, 2, C], f16, tag="qt")
            if kk % 2 == 0:
                nc.scalar.copy(qk[:], pst[:])
            else:
                nc.vector.tensor_copy(qk[:], pst[:])
            st["qt"][kk] = qk

        def emit_mm1(st, k):
            qkr = st["qt"][k // 2]
            psE = st["psE"]
            for t in range(CT):
                w = C - MVSTART[t]
                nc.tensor.matmul(
                    psE[t][:, :w],
                    qkr[:, k % 2, t * P : (t + 1) * P],
                    qkr[:, k % 2, MVSTART[t] :],
                    start=(k == 0),
                    stop=(k == KT - 1),
                )

        def emit_cast(st, s, engine):
            # fp8 cast of a q chunk: DoubleRow moving operand for mm2
            q = st["q"]
            qm = qmp.tile(
                [P, CT, FD], f8e4, tag=st["qm_tag"], bufs=st["qm_bufs"], name="qm"
            )
            src = q[:, :, s * FD : (s + 1) * FD]
            if engine == "v":
                nc.vector.tensor_copy(qm[:], src)
            else:
                nc.scalar.copy(qm[:], src)
            st["qms"][s] = qm

        def emit_mm2_t(st, s, t):
            # DoubleRow matmul pair for one (t, s) output tile
            qm = st["qms"][s]
            STdr = st["ST"]
            pu = ps_tile("pu")
            nc.tensor.matmul(
                pu[:],
                STdr[:, 0:2, t * P : (t + 1) * P],
                qm[:, 0:2, :],
                start=True,
                stop=False,
                perf_mode=DR,
            )
            nc.tensor.matmul(
                pu[:],
                STdr[:, 2:4, t * P : (t + 1) * P],
                qm[:, 2:4, :],
                start=False,
                stop=True,
                perf_mode=DR,
            )
            return pu

        def emit_epi_t(st, s, t, pu, ot, pool_add):
            # out = (U * gamma/Z) + x for one (t, s) tile
            q, grz = st["q"], st["grz"]
            xs = q[:, t, s * FD : (s + 1) * FD]
            if t % 2 == 0:
                nc.vector.scalar_tensor_tensor(
                    ot[:, t, :], pu[:], grz[t][:], xs, op0=ALU.mult, op1=ALU.add
                )
            else:
                nc.scalar.mul(ot[:, t, :], pu[:], grz[t][:])
                # pool_add: 2 = both odd tiles on PoolE (PE/ACT/DVE-dense
                # Gram window), 1 = only t=1 (store-paced tail: PoolE and
                # DVE split the adds), 0 = none
                if pool_add >= 2 or (pool_add == 1 and t == 1):
                    nc.gpsimd.tensor_add(ot[:, t, :], ot[:, t, :], xs)
                else:
                    nc.vector.tensor_add(ot[:, t, :], ot[:, t, :], xs)

        def emit_store(st, s, ot):
            nc.sync.dma_start(
                out=st["ob"][:, :, s * FD : (s + 1) * FD], in_=ot[:]
            )

        def emit_mm2_s(st, s, pool_add, split_store=False):
            # one full s-chunk: per-t matmul pair + epilogue (<=2 pu live)
            if st["qm_tag"] == "qm0":
                for c in (s, s + 1, s + 2):
                    if c < NCH and st["qms"][c] is None:
                        emit_cast(st, c, "s")
            ot = outp.tile([P, CT, FD], f32, tag="ot")
            for t in range(CT):
                pu = emit_mm2_t(st, s, t)
                emit_epi_t(st, s, t, pu, ot, pool_add)
                if split_store and t == 1:
                    # fire the first half-store as soon as tiles 0-1 are done
                    nc.sync.dma_start(
                        out=st["ob"][:, 0:2, s * FD : (s + 1) * FD],
                        in_=ot[:, 0:2, :],
                    )
            if split_store:
                nc.sync.dma_start(
                    out=st["ob"][:, 2:4, s * FD : (s + 1) * FD],
                    in_=ot[:, 2:4, :],
                )
            else:
                emit_store(st, s, ot)

        def emit_gram(st, prev, skip_chunks=0, precast=False):
            """Transposes + Gram matmul for `st`, burst-interleaved with the
            previous batch's attention-apply (mm2) so PE never idles long
            enough for the HAM clock gate to re-throttle. Batch `st`'s own
            fp8 moving-operand casts are pre-issued on VectorE here."""
            st["psE"] = [ps_tile("psE") for _ in range(CT)]
            if "qt" not in st:
                st["qt"] = [None] * (KT // 2)
            if "qh" not in st:
                st["qh"] = [None] * NCH
                emit_qh(st, 0, halves=True)
            LOOKC = 2  # chunks of pipeline distance between tr and mm1
            for kk in range(skip_chunks // 2, KT // 2):
                # fp16 cast one load-chunk ahead of the transposes
                if kk % 2 == 0:
                    for c in (kk // 2, kk // 2 + 1):
                        if c < NCH and st["qh"][c] is None:
                            emit_qh(st, c, "s" if kk % 4 else "v")
                emit_tr_pair(st, kk)
                for k in (2 * kk - LOOKC, 2 * kk + 1 - LOOKC):
                    if k >= 0:
                        emit_mm1(st, k)
                # pre-cast this batch's fp8 moving chunks (consumed by its
                # own mm2 in the kernel tail, where a cast stall would be
                # serial)
                if precast and kk % 2 == 1 and st["qms"][kk // 2] is None:
                    emit_cast(st, kk // 2, "v")
                # 6 of 8 s-groups of the previous batch's attention-apply;
                # the last two fill this batch's own softmax phase
                if prev is not None and kk % 2 == 1 and kk // 2 < NCH - 2:
                    emit_mm2_s(prev, kk // 2, pool_add=2)
            for k in (KT - 2, KT - 1):
                emit_mm1(st, k)

        def emit_softmax(st, prev=None):
            # ---- copy E out of PSUM; mirror strictly-lower blocks ----
            psE = st["psE"]
            E = []
            for t in range(CT):
                e = mats.tile([P, FD], f32, tag="E")
                w = C - MVSTART[t]
                if t % 2 == 0:
                    nc.scalar.copy(e[:, MVSTART[t] :], psE[t][:, :w])
                else:
                    nc.vector.tensor_copy(e[:, MVSTART[t] :], psE[t][:, :w])
                E.append(e)
            # row-tile 0 needs no mirrors: its rowmin goes first so exp0
            # starts while the mirrors are still being copied
            rms = [smallp.tile([P, 1], f32, tag="rm", name="rm") for _ in range(CT)]
            nc.vector.tensor_reduce(
                rms[0][:], E[0][:], axis=mybir.AxisListType.X, op=ALU.min
            )
            # E[t][:, s-block] = E[s][:, t-block].T for s < t (exact fp32
            # transposes: E magnitudes are ~4e3 and feed exp directly, so
            # low-precision rounding here would be a real error). Tile 3's
            # mirrors go first to match the exp emission order below.
            for t in (3, 2, 1):
                for s in range(t):
                    pm = ps_tile("pm")
                    nc.tensor.transpose(
                        pm[:, :P], E[s][:, t * P : (t + 1) * P], ident[:]
                    )
                    if (t + s) % 2 == 0:
                        nc.scalar.copy(E[t][:, s * P : (s + 1) * P], pm[:, :P])
                    else:
                        nc.vector.tensor_copy(
                            E[t][:, s * P : (s + 1) * P], pm[:, :P]
                        )

            # PE bubble fill: the previous batch's s-group 6 runs as bare
            # matmuls (epilogue deferred past the softmax chain so ACT/DVE
            # stay clear); for the first batch, the NEXT batch's first
            # transposes fill in instead.
            pus6 = None
            if prev is not None:
                pus6 = [emit_mm2_t(prev, NCH - 2, t) for t in range(CT)]
            elif st.get("next") is not None:
                nxt = st["next"]
                nxt["qh"] = [None] * NCH
                nxt["qt"] = [None] * (KT // 2)
                emit_qh(nxt, 0, "s")
                emit_tr_pair(nxt, 0)
                emit_qh(nxt, 1, "v")
                # this batch's first fp8 chunks: cast now so the first
                # injected attention-apply group doesn't stall on them
                emit_cast(st, 0, "s")
                emit_cast(st, 1, "v")

            # ---- suppression softmax: S = exp(rowmin - E), Z = rowsum(S),
            # pipelined per row-tile with the S transposes ----
            pstS = [
                psp.tile([P, FD], f16, tag="ps", name="pstS") for _ in range(CT)
            ]
            grz = [None] * CT
            for t in (0, 3, 2, 1):  # match mirror readiness order
                if t > 0:
                    nc.vector.tensor_reduce(
                        rms[t][:], E[t][:], axis=mybir.AxisListType.X, op=ALU.min
                    )
                s_t = mats.tile([P, FD], f16, tag="S")
                z = smallp.tile([P, 1], f32, tag="z")
                nc.scalar.activation(
                    s_t[:], E[t][:], AF.Exp, bias=rms[t][:], scale=-1.0,
                    accum_out=z[:],
                )
                for jt in range(CT):
                    nc.tensor.transpose(
                        pstS[jt][:, t * P : (t + 1) * P],
                        s_t[:, jt * P : (jt + 1) * P],
                        identh[:],
                    )
                rz = smallp.tile([P, 1], f32, tag="rz")
                nc.vector.reciprocal(rz[:], z[:])
                g = smallp.tile([P, 1], f32, tag="grz")
                nc.gpsimd.tensor_mul(g[:], rz[:], gam[:])
                grz[t] = g
            st["grz"] = grz

            # ---- STdr = S.T packed fp8_e4m3 (DoubleRow stationary) ----
            STdr = mats.tile([P, CT, C], f8e4, tag="ST", bufs=2)
            for jt in range(CT):
                if jt % 2 == 0:
                    nc.scalar.copy(STdr[:, jt, :], pstS[jt][:])
                else:
                    nc.vector.tensor_copy(STdr[:, jt, :], pstS[jt][:])
            st["ST"] = STdr

            # deferred epilogue of the bubble-fill group, then the final
            # s-group of the previous batch
            if prev is not None:
                ot6 = outp.tile([P, CT, FD], f32, tag="ot", name="ot6")
                for t in range(CT):
                    emit_epi_t(prev, NCH - 2, t, pus6[t], ot6, pool_add=2)
                emit_store(prev, NCH - 2, ot6)
                emit_mm2_s(prev, NCH - 1, pool_add=2, split_store=True)

        # ---- pipelined driver: batch b's Gram phase overlaps batch b-1's
        # attention-apply phase on the PE ----
        st0 = emit_load(0, split_first=True)
        st1 = emit_load(1)

        ident = singles.tile([P, P], f32)
        make_identity(nc, ident)
        identh = singles.tile([P, P], f16)
        nc.gpsimd.tensor_copy(identh[:], ident[:])

        # gamma broadcast to all partitions as a per-partition scalar
        gam = singles.tile([P, 1], f32)
        nc.gpsimd.dma_start(out=gam[:], in_=g_d[:].to_broadcast([P, 1]))

        # batch 0's fp8 casts ride a ScalarE lookahead ring (its mm2 is
        # interleaved into batch 1's Gram phase, so a ring is fine there);
        # batch 1's are all pre-cast during its Gram phase.
        st0["qm_tag"], st0["qm_bufs"], st0["qms"] = "qm0", 3, [None] * NCH
        st1["qm_tag"], st1["qm_bufs"], st1["qms"] = "qm1", NCH, [None] * NCH
        emit_gram(st0, None)
        st0["next"] = st1
        emit_softmax(st0, None)
        emit_gram(st1, st0, skip_chunks=2, precast=True)
        emit_softmax(st1, st0)
        for s in range(NCH):
            emit_mm2_s(st1, s, pool_add=1, split_store=True)

    nc.compile()
    return nc


def _get_nc():
    if "nc" not in _CACHE:
        _CACHE["nc"] = _build_nc()
    return _CACHE["nc"]


def kernel(x: np.ndarray, gamma: np.ndarray) -> np.ndarray:
    from concourse.bass_utils import run_bass_kernel_spmd

    nc = _get_nc()
    x = np.ascontiguousarray(np.asarray(x, dtype=np.float32))
    gamma = np.ascontiguousarray(np.asarray(gamma, dtype=np.float32))
    xs = x.reshape(B, C, N)
    in_maps = [
        {
            "x": np.ascontiguousarray(xs[c * BPC : (c + 1) * BPC]),
            "gamma": gamma,
        }
        for c in range(N_CORES)
    ]
    res = run_bass_kernel_spmd(nc, in_maps, core_ids=list(range(N_CORES)))
    out = np.stack([res.results[c]["out"] for c in range(N_CORES)], axis=0)
    return out.reshape(B, C, H, W)


# revision 23
# speedup vs baseline: 1.2537x; 1.0476x over previous
"""CAM (channel-attention) module kernel for Trainium2.

Computes, per batch b:
    q      = x[b].reshape(C, H*W)
    E      = q @ q.T                                  # [C, C] channel Gram matrix
    A[i,j] = softmax_j(rowmax_i(E) - E[i,j])          # suppression softmax
           = exp(rowmin_i(E) - E[i,j]) / Z_i
    out[b] = gamma * (A @ q) + x[b]
Distribution: pure data-parallel over batch B=16 across 8 NeuronCores
(2 batches per core); gamma replicated. No collectives.

Per-core kernel strategy (PE wall time is stream-cycles + ~45ns/inst, so
everything aims at 1-cycle/row streams and fewer, longer instructions):
  1. All 16 load DMAs (both batches) queued up front on one ring; batch
     0's first chunk is split so the first transpose starts early.
  2. q natural-layout [128, 4, 4096] exact fp32 (residual needs the
     bits); per-chunk fp16 casts qh feed the PE transposes (fp16 streams
     1 cyc/row vs fp32's two-pass LOW_HIGH mode, and keeps the same
     11-bit mantissa as float32r so the suppression argmin stays right).
  3. qT built 2 chunks per PSUM bank (fp16 halves the footprint), one
     PSUM->SBUF copy per pair; Gram matmul in fp16 with the tight
     block-triangle [512,384,256,128] (no f32r >=256 width rule).
  4. E accumulated fp32 in PSUM; strictly-lower blocks mirrored with
     exact fp32 PE transposes (E feeds exp directly).
  5. S = exp(rowmin - E) on ScalarE (bias=rowmin, scale=-1) with
     accum_out Z; S fp16. Softmax is a per-tile pipeline with almost
     nothing else contending for ACT/DVE.
  6. S transposed blockwise on PE (fp16, 1 cyc/row), packed to one
     fp8_e4m3 stationary STdr [128, jt, i] for DoubleRow.
  7. attention-apply in fp8 DoubleRow perf mode: 2 k-tiles per
     instruction halves the instruction count vs fp16. Moving operand =
     fp8 cast of q: batch 0 casts ride a lookahead ring on ScalarE;
     batch 1's are pre-cast on VectorE during its Gram phase so the
     kernel tail never waits on a cast.
  8. Epilogue out = (gamma/Z)*U + x reads exact fp32 q. Injected groups
     put one residual add on PoolE (slow but idle there); tail groups
     stay on VectorE/ScalarE.
  9. Stores batched one [128, 4, 512] DMA per s-chunk.
 10. Cross-batch pipelining: 6 of batch b-1's 8 attention-apply groups
     interleave into batch b's Gram phase; group 6's matmuls fill batch
     b's softmax PE bubble with the epilogue deferred past the softmax
     chain; group 7 runs right after the fp8 stationary is packed.
"""

import sys

import numpy as np

if "/opt/trn_rl_repo" not in sys.path:
    sys.path.insert(0, "/opt/trn_rl_repo")

B, C, H, W = 16, 512, 64, 64
N = H * W                # 4096 spatial positions
P = 128                  # partitions
CT = C // P              # 4 channel tiles
KT = N // P              # 32 contraction chunks for the Gram matmul
FD = 512                 # matmul moving free dim / PSUM bank width (fp32)
NCH = N // FD            # 8 output column chunks
N_CORES = 8
BPC = B // N_CORES       # 2 batches per core

# Moving-operand start column for the upper-triangular Gram matmul (fp16
# streams 1 cyc/row at any width, so the triangle is exact).
MVSTART = [0, 128, 256, 384]

_CACHE = {}


def _build_nc():
    from contextlib import ExitStack

    import concourse.bacc as bacc
    import concourse.tile as tile
    from concourse import mybir
    from concourse.masks import make_identity

    f32 = mybir.dt.float32
    f16 = mybir.dt.float16
    f8e4 = mybir.dt.float8e4
    AF = mybir.ActivationFunctionType
    ALU = mybir.AluOpType
    DR = mybir.MatmulPerfMode.DoubleRow

    nc = bacc.Bacc(None, target_bir_lowering=False)
    # x stays float32 end-to-end on the load path (the DMA cast unit would
    # round); reduced-precision PE operands come from engine casts.
    x_d = nc.dram_tensor("x", [BPC, C, N], f32, kind="ExternalInput")
    g_d = nc.dram_tensor("gamma", [1], f32, kind="ExternalInput")
    o_d = nc.dram_tensor("out", [BPC, C, N], f32, kind="ExternalOutput")

    with ExitStack() as ctx:
        tc = ctx.enter_context(tile.TileContext(nc))
        singles = ctx.enter_context(tc.tile_pool(name="singles", bufs=1))
        bigq = ctx.enter_context(tc.tile_pool(name="bigq", bufs=2))
        qhp = ctx.enter_context(tc.tile_pool(name="qhp", bufs=3))
        qtp = ctx.enter_context(tc.tile_pool(name="qtp", bufs=3))
        qmp = ctx.enter_context(tc.tile_pool(name="qmp", bufs=2))
        mats = ctx.enter_context(tc.tile_pool(name="mats", bufs=4))
        outp = ctx.enter_context(tc.tile_pool(name="outp", bufs=2))
        smallp = ctx.enter_context(tc.tile_pool(name="small", bufs=8))
        psp = ctx.enter_context(tc.tile_pool(name="ps", bufs=8, space="PSUM"))

        def ps_tile(name="ps"):
            return psp.tile([P, FD], f32, tag="ps", name=name)

        def emit_load(b, split_first=False):
            xb = x_d[b].rearrange("(ct p) n -> p ct n", p=P)
            ob = o_d[b].rearrange("(ct p) n -> p ct n", p=P)
            q = bigq.tile([P, CT, N], f32, tag="q")
            for s in range(NCH):
                if split_first and s == 0:
                    h = FD // 2
                    nc.sync.dma_start(out=q[:, :, 0:h], in_=xb[:, :, 0:h])
                    nc.sync.dma_start(out=q[:, :, h:FD], in_=xb[:, :, h:FD])
                    continue
                nc.sync.dma_start(
                    out=q[:, :, s * FD : (s + 1) * FD],
                    in_=xb[:, :, s * FD : (s + 1) * FD],
                )
            return {"q": q, "xb": xb, "ob": ob}

        def emit_qh(st, c, engine="s", halves=False):
            # fp16 cast of q chunk c (feeds the PE transposes)
            q = st["q"]
            qh = qhp.tile([P, CT, FD], f16, tag="qh", name="qh")
            if halves:
                # first transposes only need the first half: don't wait for
                # the whole chunk to land
                h = FD // 2
                nc.scalar.copy(qh[:, :, 0:h], q[:, :, c * FD : c * FD + h])
                nc.scalar.copy(qh[:, :, h:FD], q[:, :, c * FD + h : (c + 1) * FD])
            elif engine == "v":
                nc.vector.tensor_copy(qh[:], q[:, :, c * FD : (c + 1) * FD])
            else:
                nc.scalar.copy(qh[:], q[:, :, c * FD : (c + 1) * FD])
            st["qh"][c] = qh

        def emit_tr_pair(st, kk):
            # transpose chunks 2kk and 2kk+1 into one fp16 PSUM bank;
            # a single PSUM->SBUF copy yields both Gram operand chunks
            pst = psp.tile([P, 2, FD], f16, tag="ps", name="pstr")
            for i in range(2):
                k = 2 * kk + i
                qh = st["qh"][k // 4]
                for t in range(CT):
                    nc.tensor.transpose(
                        pst[:, i, t * P : (t + 1) * P],
                        qh[:, t, (k % 4) * P : (k % 4 + 1) * P],
                        identh[:],
                    )
            qk = qtp.tile([P, 2, C], f16, tag="qt")
            if kk % 2 == 0:
                nc.scalar.copy(qk[:], pst[:])
            else:
                nc.vector.tensor_copy(qk[:], pst[:])
            st["qt"][kk] = qk

        def emit_mm1(st, k):
            qkr = st["qt"][k // 2]
            psE = st["psE"]
            for t in range(CT):
                w = C - MVSTART[t]
                nc.tensor.matmul(
                    psE[t][:, :w],
                    qkr[:, k % 2, t * P : (t + 1) * P],
                    qkr[:, k % 2, MVSTART[t] :],
                    start=(k == 0),
                    stop=(k == KT - 1),
                )

        def emit_cast(st, s, engine):
            # fp8 cast of a q chunk: DoubleRow moving operand for mm2
            q = st["q"]
            qm = qmp.tile(
                [P, CT, FD], f8e4, tag=st["qm_tag"], bufs=st["qm_bufs"], name="qm"
            )
            src = q[:, :, s * FD : (s + 1) * FD]
            if engine == "v":
                nc.vector.tensor_copy(qm[:], src)
            else:
                nc.scalar.copy(qm[:], src)
            st["qms"][s] = qm

        def emit_mm2_t(st, s, t):
            # DoubleRow matmul pair for one (t, s) output tile
            qm = st["qms"][s]
            STdr = st["ST"]
            pu = ps_tile("pu")
            nc.tensor.matmul(
                pu[:],
                STdr[:, 0:2, t * P : (t + 1) * P],
                qm[:, 0:2, :],
                start=True,
                stop=False,
                perf_mode=DR,
            )
            nc.tensor.matmul(
                pu[:],
                STdr[:, 2:4, t * P : (t + 1) * P],
                qm[:, 2:4, :],
                start=False,
                stop=True,
                perf_mode=DR,
            )
            return pu

        def emit_epi_t(st, s, t, pu, ot, pool_add):
            # out = (U * gamma/Z) + x for one (t, s) tile
            q, grz = st["q"], st["grz"]
            xs = q[:, t, s * FD : (s + 1) * FD]
            if t % 2 == 0:
                nc.vector.scalar_tensor_tensor(
                    ot[:, t, :], pu[:], grz[t][:], xs, op0=ALU.mult, op1=ALU.add
                )
            else:
                nc.scalar.mul(ot[:, t, :], pu[:], grz[t][:])
                # pool_add: 2 = both odd tiles on PoolE (PE/ACT/DVE-dense
                # Gram window), 1 = only t=1 (store-paced tail: PoolE and
                # DVE split the adds), 0 = none
                if pool_add >= 2 or (pool_add == 1 and t == 1):
                    nc.gpsimd.tensor_add(ot[:, t, :], ot[:, t, :], xs)
                else:
                    nc.vector.tensor_add(ot[:, t, :], ot[:, t, :], xs)

        def emit_store(st, s, ot):
            nc.sync.dma_start(
                out=st["ob"][:, :, s * FD : (s + 1) * FD], in_=ot[:]
            )

        def emit_mm2_s(st, s, pool_add, split_store=False):
            # one full s-chunk: per-t matmul pair + epilogue (<=2 pu live)
            if st["qm_tag"] == "qm0":
                for c in (s, s + 1, s + 2):
                    if c < NCH and st["qms"][c] is None:
                        emit_cast(st, c, "s")
            ot = outp.tile([P, CT, FD], f32, tag="ot")
            for t in range(CT):
                pu = emit_mm2_t(st, s, t)
                emit_epi_t(st, s, t, pu, ot, pool_add)
                if split_store and t == 1:
                    # fire the first half-store as soon as tiles 0-1 are done
                    nc.sync.dma_start(
                        out=st["ob"][:, 0:2, s * FD : (s + 1) * FD],
                        in_=ot[:, 0:2, :],
                    )
            if split_store:
                nc.sync.dma_start(
                    out=st["ob"][:, 2:4, s * FD : (s + 1) * FD],
                    in_=ot[:, 2:4, :],
                )
            else:
                emit_store(st, s, ot)

        def emit_gram(st, prev, skip_chunks=0, precast=False):
            """Transposes + Gram matmul for `st`, burst-interleaved with the
            previous batch's attention-apply (mm2) so PE never idles long
            enough for the HAM clock gate to re-throttle. Batch `st`'s own
            fp8 moving-operand casts are pre-issued on VectorE here."""
            st["psE"] = [ps_tile("psE") for _ in range(CT)]
            if "qt" not in st:
                st["qt"] = [None] * (KT // 2)
            if "qh" not in st:
                st["qh"] = [None] * NCH
                emit_qh(st, 0, halves=True)
            LOOKC = 2  # chunks of pipeline distance between tr and mm1
            for kk in range(skip_chunks // 2, KT // 2):
                # fp16 cast one load-chunk ahead of the transposes
                if kk % 2 == 0:
                    for c in (kk // 2, kk // 2 + 1):
                        if c < NCH and st["qh"][c] is None:
                            emit_qh(st, c, "s" if kk % 4 else "v")
                emit_tr_pair(st, kk)
                for k in (2 * kk - LOOKC, 2 * kk + 1 - LOOKC):
                    if k >= 0:
                        emit_mm1(st, k)
                # pre-cast this batch's fp8 moving chunks (consumed by its
                # own mm2 in the kernel tail, where a cast stall would be
                # serial)
                if precast and kk % 2 == 1 and st["qms"][kk // 2] is None:
                    emit_cast(st, kk // 2, "v")
                # 6 of 8 s-groups of the previous batch's attention-apply;
                # the last two fill this batch's own softmax phase
                if prev is not None and kk % 2 == 1 and kk // 2 < NCH - 2:
                    emit_mm2_s(prev, kk // 2, pool_add=1)
            for k in (KT - 2, KT - 1):
                emit_mm1(st, k)

        def emit_softmax(st, prev=None):
            # ---- copy E out of PSUM; mirror strictly-lower blocks ----
            psE = st["psE"]
            E = []
            for t in range(CT):
                e = mats.tile([P, FD], f32, tag="E")
                w = C - MVSTART[t]
                if t % 2 == 0:
                    nc.scalar.copy(e[:, MVSTART[t] :], psE[t][:, :w])
                else:
                    nc.vector.tensor_copy(e[:, MVSTART[t] :], psE[t][:, :w])
                E.append(e)
            # row-tile 0 needs no mirrors: its rowmin goes first so exp0
            # starts while the mirrors are still being copied
            rms = [smallp.tile([P, 1], f32, tag="rm", name="rm") for _ in range(CT)]
            nc.vector.tensor_reduce(
                rms[0][:], E[0][:], axis=mybir.AxisListType.X, op=ALU.min
            )
            # E[t][:, s-block] = E[s][:, t-block].T for s < t (exact fp32
            # transposes: E magnitudes are ~4e3 and feed exp directly, so
            # low-precision rounding here would be a real error). Tile 3's
            # mirrors go first to match the exp emission order below.
            for t in (3, 2, 1):
                for s in range(t):
                    pm = ps_tile("pm")
                    nc.tensor.transpose(
                        pm[:, :P], E[s][:, t * P : (t + 1) * P], ident[:]
                    )
                    if (t + s) % 2 == 0:
                        nc.scalar.copy(E[t][:, s * P : (s + 1) * P], pm[:, :P])
                    else:
                        nc.vector.tensor_copy(
                            E[t][:, s * P : (s + 1) * P], pm[:, :P]
                        )

            # PE bubble fill: the previous batch's s-group 6 runs as bare
            # matmuls (epilogue deferred past the softmax chain so ACT/DVE
            # stay clear); for the first batch, the NEXT batch's first
            # transposes fill in instead.
            pus6 = None
            if prev is not None:
                pus6 = [emit_mm2_t(prev, NCH - 2, t) for t in range(CT)]
            elif st.get("next") is not None:
                nxt = st["next"]
                nxt["qh"] = [None] * NCH
                nxt["qt"] = [None] * (KT // 2)
                emit_qh(nxt, 0, "s")
                emit_tr_pair(nxt, 0)
                emit_qh(nxt, 1, "v")
                # this batch's first fp8 chunks: cast now so the first
                # injected attention-apply group doesn't stall on them
                emit_cast(st, 0, "s")
                emit_cast(st, 1, "v")

            # ---- suppression softmax: S = exp(rowmin - E), Z = rowsum(S),
            # pipelined per row-tile with the S transposes ----
            pstS = [
                psp.tile([P, FD], f16, tag="ps", name="pstS") for _ in range(CT)
            ]
            grz = [None] * CT
            for t in (0, 3, 2, 1):  # match mirror readiness order
                if t > 0:
                    nc.vector.tensor_reduce(
                        rms[t][:], E[t][:], axis=mybir.AxisListType.X, op=ALU.min
                    )
                s_t = mats.tile([P, FD], f16, tag="S")
                z = smallp.tile([P, 1], f32, tag="z")
                nc.scalar.activation(
                    s_t[:], E[t][:], AF.Exp, bias=rms[t][:], scale=-1.0,
                    accum_out=z[:],
                )
                for jt in range(CT):
                    nc.tensor.transpose(
                        pstS[jt][:, t * P : (t + 1) * P],
                        s_t[:, jt * P : (jt + 1) * P],
                        identh[:],
                    )
                rz = smallp.tile([P, 1], f32, tag="rz")
                nc.vector.reciprocal(rz[:], z[:])
                g = smallp.tile([P, 1], f32, tag="grz")
                nc.gpsimd.tensor_mul(g[:], rz[:], gam[:])
                grz[t] = g
            st["grz"] = grz

            # ---- STdr = S.T packed fp8_e4m3 (DoubleRow stationary) ----
            STdr = mats.tile([P, CT, C], f8e4, tag="ST", bufs=2)
            for jt in range(CT):
                if jt % 2 == 0:
                    nc.scalar.copy(STdr[:, jt, :], pstS[jt][:])
                else:
                    nc.vector.tensor_copy(STdr[:, jt, :], pstS[jt][:])
            st["ST"] = STdr

            # deferred epilogue of the bubble-fill group, then the final
            # s-group of the previous batch
            if prev is not None:
                ot6 = outp.tile([P, CT, FD], f32, tag="ot", name="ot6")
                for t in range(CT):
                    emit_epi_t(prev, NCH - 2, t, pus6[t], ot6, pool_add=1)
                emit_store(prev, NCH - 2, ot6)
                emit_mm2_s(prev, NCH - 1, pool_add=1, split_store=True)

        # ---- pipelined driver: batch b's Gram phase overlaps batch b-1's
        # attention-apply phase on the PE ----
        st0 = emit_load(0, split_first=True)
        st1 = emit_load(1)

        ident = singles.tile([P, P], f32)
        make_identity(nc, ident)
        identh = singles.tile([P, P], f16)
        nc.gpsimd.tensor_copy(identh[:], ident[:])

        # gamma broadcast to all partitions as a per-partition scalar
        gam = singles.tile([P, 1], f32)
        nc.gpsimd.dma_start(out=gam[:], in_=g_d[:].to_broadcast([P, 1]))

        # batch 0's fp8 casts ride a ScalarE lookahead ring (its mm2 is
        # interleaved into batch 1's Gram phase, so a ring is fine there);
        # batch 1's are all pre-cast during its Gram phase.
        st0["qm_tag"], st0["qm_bufs"], st0["qms"] = "qm0", 3, [None] * NCH
        st1["qm_tag"], st1["qm_bufs"], st1["qms"] = "qm1", NCH, [None] * NCH
        emit_gram(st0, None)
        st0["next"] = st1
        emit_softmax(st0, None)
        emit_gram(st1, st0, skip_chunks=2, precast=True)
        emit_softmax(st1, st0)
        for s in range(NCH):
            emit_mm2_s(st1, s, pool_add=0, split_store=True)

    nc.compile()
    return nc


def _get_nc():
    if "nc" not in _CACHE:
        _CACHE["nc"] = _build_nc()
    return _CACHE["nc"]


def kernel(x: np.ndarray, gamma: np.ndarray) -> np.ndarray:
    from concourse.bass_utils import run_bass_kernel_spmd

    nc = _get_nc()
    x = np.ascontiguousarray(np.asarray(x, dtype=np.float32))
    gamma = np.ascontiguousarray(np.asarray(gamma, dtype=np.float32))
    xs = x.reshape(B, C, N)
    in_maps = [
        {
            "x": np.ascontiguousarray(xs[c * BPC : (c + 1) * BPC]),
            "gamma": gamma,
        }
        for c in range(N_CORES)
    ]
    res = run_bass_kernel_spmd(nc, in_maps, core_ids=list(range(N_CORES)))
    out = np.stack([res.results[c]["out"] for c in range(N_CORES)], axis=0)
    return out.reshape(B, C, H, W)


# revision 26
# speedup vs baseline: 1.2721x; 1.0147x over previous
"""CAM (channel-attention) module kernel for Trainium2.

Computes, per batch b:
    q      = x[b].reshape(C, H*W)
    E      = q @ q.T                                  # [C, C] channel Gram matrix
    A[i,j] = softmax_j(rowmax_i(E) - E[i,j])          # suppression softmax
           = exp(rowmin_i(E) - E[i,j]) / Z_i
    out[b] = gamma * (A @ q) + x[b]
Distribution: pure data-parallel over batch B=16 across 8 NeuronCores
(2 batches per core); gamma replicated. No collectives.

Per-core kernel strategy (PE wall time is stream-cycles + ~45ns/inst, so
everything aims at 1-cycle/row streams and fewer, longer instructions):
  1. All 16 load DMAs (both batches) queued up front on one ring; batch
     0's first chunk is split so the first transpose starts early.
  2. q natural-layout [128, 4, 4096] exact fp32 (residual needs the
     bits); per-chunk fp16 casts qh feed the PE transposes (fp16 streams
     1 cyc/row vs fp32's two-pass LOW_HIGH mode, and keeps the same
     11-bit mantissa as float32r so the suppression argmin stays right).
  3. qT built 2 chunks per PSUM bank (fp16 halves the footprint), one
     PSUM->SBUF copy per pair; Gram matmul in fp16 with the tight
     block-triangle [512,384,256,128] (no f32r >=256 width rule).
  4. E accumulated fp32 in PSUM; strictly-lower blocks mirrored with
     exact fp32 PE transposes (E feeds exp directly).
  5. S = exp(rowmin - E) on ScalarE (bias=rowmin, scale=-1) with
     accum_out Z; S fp16. Softmax is a per-tile pipeline with almost
     nothing else contending for ACT/DVE.
  6. S transposed blockwise on PE (fp16, 1 cyc/row), packed to one
     fp8_e4m3 stationary STdr [128, jt, i] for DoubleRow.
  7. attention-apply in fp8 DoubleRow perf mode: 2 k-tiles per
     instruction halves the instruction count vs fp16. Moving operand =
     fp8 cast of q: batch 0 casts ride a lookahead ring on ScalarE;
     batch 1's are pre-cast on VectorE during its Gram phase so the
     kernel tail never waits on a cast.
  8. Epilogue out = (gamma/Z)*U + x reads exact fp32 q. Injected groups
     put one residual add on PoolE (slow but idle there); tail groups
     stay on VectorE/ScalarE.
  9. Stores batched one [128, 4, 512] DMA per s-chunk.
 10. Cross-batch pipelining: 6 of batch b-1's 8 attention-apply groups
     interleave into batch b's Gram phase; group 6's matmuls fill batch
     b's softmax PE bubble with the epilogue deferred past the softmax
     chain; group 7 runs right after the fp8 stationary is packed.
"""

import sys

import numpy as np

if "/opt/trn_rl_repo" not in sys.path:
    sys.path.insert(0, "/opt/trn_rl_repo")

B, C, H, W = 16, 512, 64, 64
N = H * W                # 4096 spatial positions
P = 128                  # partitions
CT = C // P              # 4 channel tiles
KT = N // P              # 32 contraction chunks for the Gram matmul
FD = 512                 # matmul moving free dim / PSUM bank width (fp32)
NCH = N // FD            # 8 output column chunks
N_CORES = 8
BPC = B // N_CORES       # 2 batches per core

# Moving-operand start column for the upper-triangular Gram matmul (fp16
# streams 1 cyc/row at any width, so the triangle is exact).
MVSTART = [0, 128, 256, 384]

_CACHE = {}


def _build_nc():
    from contextlib import ExitStack

    import concourse.bacc as bacc
    import concourse.tile as tile
    from concourse import mybir
    from concourse.masks import make_identity

    f32 = mybir.dt.float32
    f16 = mybir.dt.float16
    f8e4 = mybir.dt.float8e4
    AF = mybir.ActivationFunctionType
    ALU = mybir.AluOpType
    DR = mybir.MatmulPerfMode.DoubleRow

    nc = bacc.Bacc(None, target_bir_lowering=False)
    # x stays float32 end-to-end on the load path (the DMA cast unit would
    # round); reduced-precision PE operands come from engine casts.
    x_d = nc.dram_tensor("x", [BPC, C, N], f32, kind="ExternalInput")
    g_d = nc.dram_tensor("gamma", [1], f32, kind="ExternalInput")
    o_d = nc.dram_tensor("out", [BPC, C, N], f32, kind="ExternalOutput")

    with ExitStack() as ctx:
        tc = ctx.enter_context(tile.TileContext(nc))
        singles = ctx.enter_context(tc.tile_pool(name="singles", bufs=1))
        bigq = ctx.enter_context(tc.tile_pool(name="bigq", bufs=2))
        qhp = ctx.enter_context(tc.tile_pool(name="qhp", bufs=2))
        qtp = ctx.enter_context(tc.tile_pool(name="qtp", bufs=4))
        qmp = ctx.enter_context(tc.tile_pool(name="qmp", bufs=2))
        mats = ctx.enter_context(tc.tile_pool(name="mats", bufs=4))
        outp = ctx.enter_context(tc.tile_pool(name="outp", bufs=3))
        smallp = ctx.enter_context(tc.tile_pool(name="small", bufs=8))
        psp = ctx.enter_context(tc.tile_pool(name="ps", bufs=8, space="PSUM"))

        def ps_tile(name="ps"):
            return psp.tile([P, FD], f32, tag="ps", name=name)

        def emit_load(b, split_first=False):
            xb = x_d[b].rearrange("(ct p) n -> p ct n", p=P)
            ob = o_d[b].rearrange("(ct p) n -> p ct n", p=P)
            q = bigq.tile([P, CT, N], f32, tag="q")
            for s in range(NCH):
                if split_first and s == 0:
                    h = FD // 2
                    nc.sync.dma_start(out=q[:, :, 0:h], in_=xb[:, :, 0:h])
                    nc.sync.dma_start(out=q[:, :, h:FD], in_=xb[:, :, h:FD])
                    continue
                nc.sync.dma_start(
                    out=q[:, :, s * FD : (s + 1) * FD],
                    in_=xb[:, :, s * FD : (s + 1) * FD],
                )
            return {"q": q, "xb": xb, "ob": ob}

        def emit_qh(st, c, engine="s", halves=False):
            # fp16 cast of q chunk c (feeds the PE transposes)
            q = st["q"]
            qh = qhp.tile([P, CT, FD], f16, tag="qh", name="qh")
            if halves:
                # first transposes only need the first half: don't wait for
                # the whole chunk to land
                h = FD // 2
                nc.scalar.copy(qh[:, :, 0:h], q[:, :, c * FD : c * FD + h])
                nc.scalar.copy(qh[:, :, h:FD], q[:, :, c * FD + h : (c + 1) * FD])
            elif engine == "v":
                nc.vector.tensor_copy(qh[:], q[:, :, c * FD : (c + 1) * FD])
            else:
                nc.scalar.copy(qh[:], q[:, :, c * FD : (c + 1) * FD])
            st["qh"][c] = qh

        def emit_tr_pair(st, kk):
            # transpose chunks 2kk and 2kk+1 into one fp16 PSUM bank;
            # a single PSUM->SBUF copy yields both Gram operand chunks
            pst = psp.tile([P, 2, FD], f16, tag="ps", name="pstr")
            for i in range(2):
                k = 2 * kk + i
                qh = st["qh"][k // 4]
                for t in range(CT):
                    nc.tensor.transpose(
                        pst[:, i, t * P : (t + 1) * P],
                        qh[:, t, (k % 4) * P : (k % 4 + 1) * P],
                        identh[:],
                    )
            qk = qtp.tile([P, 2, C], f16, tag="qt")
            if kk % 2 == 0:
                nc.scalar.copy(qk[:], pst[:])
            else:
                nc.vector.tensor_copy(qk[:], pst[:])
            st["qt"][kk] = qk

        def emit_mm1(st, k):
            qkr = st["qt"][k // 2]
            psE = st["psE"]
            for t in range(CT):
                w = C - MVSTART[t]
                nc.tensor.matmul(
                    psE[t][:, :w],
                    qkr[:, k % 2, t * P : (t + 1) * P],
                    qkr[:, k % 2, MVSTART[t] :],
                    start=(k == 0),
                    stop=(k == KT - 1),
                )

        def emit_cast(st, s, engine):
            # fp8 cast of a q chunk: DoubleRow moving operand for mm2
            q = st["q"]
            qm = qmp.tile(
                [P, CT, FD], f8e4, tag=st["qm_tag"], bufs=st["qm_bufs"], name="qm"
            )
            src = q[:, :, s * FD : (s + 1) * FD]
            if engine == "v":
                nc.vector.tensor_copy(qm[:], src)
            else:
                nc.scalar.copy(qm[:], src)
            st["qms"][s] = qm

        def emit_mm2_t(st, s, t):
            # DoubleRow matmul pair for one (t, s) output tile
            qm = st["qms"][s]
            STdr = st["ST"]
            pu = ps_tile("pu")
            nc.tensor.matmul(
                pu[:],
                STdr[:, 0:2, t * P : (t + 1) * P],
                qm[:, 0:2, :],
                start=True,
                stop=False,
                perf_mode=DR,
            )
            nc.tensor.matmul(
                pu[:],
                STdr[:, 2:4, t * P : (t + 1) * P],
                qm[:, 2:4, :],
                start=False,
                stop=True,
                perf_mode=DR,
            )
            return pu

        def emit_epi_t(st, s, t, pu, ot, pool_add):
            # out = (U * gamma/Z) + x for one (t, s) tile
            q, grz = st["q"], st["grz"]
            xs = q[:, t, s * FD : (s + 1) * FD]
            if t % 2 == 0:
                nc.vector.scalar_tensor_tensor(
                    ot[:, t, :], pu[:], grz[t][:], xs, op0=ALU.mult, op1=ALU.add
                )
            else:
                nc.scalar.mul(ot[:, t, :], pu[:], grz[t][:])
                # pool_add: 2 = both odd tiles on PoolE (PE/ACT/DVE-dense
                # Gram window), 1 = only t=1 (store-paced tail: PoolE and
                # DVE split the adds), 0 = none
                if pool_add >= 2 or (pool_add == 1 and t == 1):
                    nc.gpsimd.tensor_add(ot[:, t, :], ot[:, t, :], xs)
                else:
                    nc.vector.tensor_add(ot[:, t, :], ot[:, t, :], xs)

        def emit_store(st, s, ot):
            nc.sync.dma_start(
                out=st["ob"][:, :, s * FD : (s + 1) * FD], in_=ot[:]
            )

        def emit_mm2_s(st, s, pool_add, split_store=False):
            # one full s-chunk: per-t matmul pair + epilogue (<=2 pu live)
            if st["qm_tag"] == "qm0":
                for c in (s, s + 1, s + 2):
                    if c < NCH and st["qms"][c] is None:
                        emit_cast(st, c, "s")
            ot = outp.tile([P, CT, FD], f32, tag="ot")
            for t in range(CT):
                pu = emit_mm2_t(st, s, t)
                emit_epi_t(st, s, t, pu, ot, pool_add)
                if split_store and t == 1:
                    # fire the first half-store as soon as tiles 0-1 are done
                    nc.sync.dma_start(
                        out=st["ob"][:, 0:2, s * FD : (s + 1) * FD],
                        in_=ot[:, 0:2, :],
                    )
            if split_store:
                nc.sync.dma_start(
                    out=st["ob"][:, 2:4, s * FD : (s + 1) * FD],
                    in_=ot[:, 2:4, :],
                )
            else:
                emit_store(st, s, ot)

        def emit_gram(st, prev, skip_chunks=0, precast=False):
            """Transposes + Gram matmul for `st`, burst-interleaved with the
            previous batch's attention-apply (mm2) so PE never idles long
            enough for the HAM clock gate to re-throttle. Batch `st`'s own
            fp8 moving-operand casts are pre-issued on VectorE here."""
            st["psE"] = [ps_tile("psE") for _ in range(CT)]
            if "qt" not in st:
                st["qt"] = [None] * (KT // 2)
            if "qh" not in st:
                st["qh"] = [None] * NCH
                emit_qh(st, 0, halves=True)
            LOOKC = 2  # chunks of pipeline distance between tr and mm1
            for kk in range(skip_chunks // 2, KT // 2):
                # fp16 cast one load-chunk ahead of the transposes
                if kk % 2 == 0:
                    for c in (kk // 2, kk // 2 + 1):
                        if c < NCH and st["qh"][c] is None:
                            emit_qh(st, c, "s" if kk % 4 else "v")
                emit_tr_pair(st, kk)
                for k in (2 * kk - LOOKC, 2 * kk + 1 - LOOKC):
                    if k >= 0:
                        emit_mm1(st, k)
                # pre-cast this batch's fp8 moving chunks (consumed by its
                # own mm2 in the kernel tail, where a cast stall would be
                # serial)
                if precast and kk % 2 == 1 and st["qms"][kk // 2] is None:
                    emit_cast(st, kk // 2, "v")
                # 6 of 8 s-groups of the previous batch's attention-apply;
                # the last two fill this batch's own softmax phase
                if prev is not None and kk % 2 == 1 and kk // 2 < NCH - 2:
                    emit_mm2_s(prev, kk // 2, pool_add=1)
            for k in (KT - 2, KT - 1):
                emit_mm1(st, k)

        def emit_softmax(st, prev=None):
            # ---- copy E out of PSUM; mirror strictly-lower blocks ----
            psE = st["psE"]
            E = []
            for t in range(CT):
                e = mats.tile([P, FD], f32, tag="E")
                w = C - MVSTART[t]
                if t % 2 == 0:
                    nc.scalar.copy(e[:, MVSTART[t] :], psE[t][:, :w])
                else:
                    nc.vector.tensor_copy(e[:, MVSTART[t] :], psE[t][:, :w])
                E.append(e)
            # row-tile 0 needs no mirrors: its rowmin goes first so exp0
            # starts while the mirrors are still being copied
            rms = [smallp.tile([P, 1], f32, tag="rm", name="rm") for _ in range(CT)]
            nc.vector.tensor_reduce(
                rms[0][:], E[0][:], axis=mybir.AxisListType.X, op=ALU.min
            )
            # E[t][:, s-block] = E[s][:, t-block].T for s < t (exact fp32
            # transposes: E magnitudes are ~4e3 and feed exp directly, so
            # low-precision rounding here would be a real error). Tile 3's
            # mirrors go first to match the exp emission order below.
            for t in (3, 2, 1):
                for s in range(t):
                    pm = ps_tile("pm")
                    nc.tensor.transpose(
                        pm[:, :P], E[s][:, t * P : (t + 1) * P], ident[:]
                    )
                    if (t + s) % 2 == 0:
                        nc.scalar.copy(E[t][:, s * P : (s + 1) * P], pm[:, :P])
                    else:
                        nc.vector.tensor_copy(
                            E[t][:, s * P : (s + 1) * P], pm[:, :P]
                        )

            # PE bubble fill: the previous batch's s-group 6 runs as bare
            # matmuls (epilogue deferred past the softmax chain so ACT/DVE
            # stay clear); for the first batch, the NEXT batch's first
            # transposes fill in instead.
            pus6 = None
            if prev is not None:
                pus6 = [emit_mm2_t(prev, NCH - 2, t) for t in range(CT)]
            elif st.get("next") is not None:
                nxt = st["next"]
                nxt["qh"] = [None] * NCH
                nxt["qt"] = [None] * (KT // 2)
                emit_qh(nxt, 0, "s")
                emit_tr_pair(nxt, 0)
                emit_qh(nxt, 1, "v")
                # this batch's first fp8 chunks: cast now so the first
                # injected attention-apply group doesn't stall on them
                emit_cast(st, 0, "s")
                emit_cast(st, 1, "v")

            # ---- suppression softmax: S = exp(rowmin - E), Z = rowsum(S),
            # pipelined per row-tile with the S transposes ----
            pstS = [
                psp.tile([P, FD], f16, tag="ps", name="pstS") for _ in range(CT)
            ]
            grz = [None] * CT
            for t in (0, 3, 2, 1):  # match mirror readiness order
                if t > 0:
                    nc.vector.tensor_reduce(
                        rms[t][:], E[t][:], axis=mybir.AxisListType.X, op=ALU.min
                    )
                s_t = mats.tile([P, FD], f16, tag="S")
                z = smallp.tile([P, 1], f32, tag="z")
                nc.scalar.activation(
                    s_t[:], E[t][:], AF.Exp, bias=rms[t][:], scale=-1.0,
                    accum_out=z[:],
                )
                for jt in range(CT):
                    nc.tensor.transpose(
                        pstS[jt][:, t * P : (t + 1) * P],
                        s_t[:, jt * P : (jt + 1) * P],
                        identh[:],
                    )
                rz = smallp.tile([P, 1], f32, tag="rz")
                nc.vector.reciprocal(rz[:], z[:])
                g = smallp.tile([P, 1], f32, tag="grz")
                nc.gpsimd.tensor_mul(g[:], rz[:], gam[:])
                grz[t] = g
            st["grz"] = grz

            # ---- STdr = S.T packed fp8_e4m3 (DoubleRow stationary) ----
            STdr = mats.tile([P, CT, C], f8e4, tag="ST", bufs=2)
            for jt in range(CT):
                if jt % 2 == 0:
                    nc.scalar.copy(STdr[:, jt, :], pstS[jt][:])
                else:
                    nc.vector.tensor_copy(STdr[:, jt, :], pstS[jt][:])
            st["ST"] = STdr

            # deferred epilogue of the bubble-fill group, then the final
            # s-group of the previous batch
            if prev is not None:
                ot6 = outp.tile([P, CT, FD], f32, tag="ot", name="ot6")
                for t in range(CT):
                    emit_epi_t(prev, NCH - 2, t, pus6[t], ot6, pool_add=1)
                emit_store(prev, NCH - 2, ot6)
                emit_mm2_s(prev, NCH - 1, pool_add=1, split_store=True)

        # ---- pipelined driver: batch b's Gram phase overlaps batch b-1's
        # attention-apply phase on the PE ----
        st0 = emit_load(0, split_first=True)
        st1 = emit_load(1)

        ident = singles.tile([P, P], f32)
        make_identity(nc, ident)
        identh = singles.tile([P, P], f16)
        nc.gpsimd.tensor_copy(identh[:], ident[:])

        # gamma broadcast to all partitions as a per-partition scalar
        gam = singles.tile([P, 1], f32)
        nc.gpsimd.dma_start(out=gam[:], in_=g_d[:].to_broadcast([P, 1]))

        # batch 0's fp8 casts ride a ScalarE lookahead ring (its mm2 is
        # interleaved into batch 1's Gram phase, so a ring is fine there);
        # batch 1's are all pre-cast during its Gram phase.
        st0["qm_tag"], st0["qm_bufs"], st0["qms"] = "qm0", 3, [None] * NCH
        st1["qm_tag"], st1["qm_bufs"], st1["qms"] = "qm1", NCH, [None] * NCH
        emit_gram(st0, None)
        st0["next"] = st1
        emit_softmax(st0, None)
        emit_gram(st1, st0, skip_chunks=2, precast=True)
        emit_softmax(st1, st0)
        for s in range(NCH):
            emit_mm2_s(st1, s, pool_add=0, split_store=True)

    nc.compile()
    return nc


def _get_nc():
    if "nc" not in _CACHE:
        _CACHE["nc"] = _build_nc()
    return _CACHE["nc"]


def kernel(x: np.ndarray, gamma: np.ndarray) -> np.ndarray:
    from concourse.bass_utils import run_bass_kernel_spmd

    nc = _get_nc()
    x = np.ascontiguousarray(np.asarray(x, dtype=np.float32))
    gamma = np.ascontiguousarray(np.asarray(gamma, dtype=np.float32))
    xs = x.reshape(B, C, N)
    in_maps = [
        {
            "x": np.ascontiguousarray(xs[c * BPC : (c + 1) * BPC]),
            "gamma": gamma,
        }
        for c in range(N_CORES)
    ]
    res = run_bass_kernel_spmd(nc, in_maps, core_ids=list(range(N_CORES)))
    out = np.stack([res.results[c]["out"] for c in range(N_CORES)], axis=0)
    return out.reshape(B, C, H, W)


# revision 32
# speedup vs baseline: 1.3257x; 1.0421x over previous
"""CAM (channel-attention) module kernel for Trainium2.

Computes, per batch b:
    q      = x[b].reshape(C, H*W)
    E      = q @ q.T                                  # [C, C] channel Gram matrix
    A[i,j] = softmax_j(rowmax_i(E) - E[i,j])          # suppression softmax
           = exp(rowmin_i(E) - E[i,j]) / Z_i
    out[b] = gamma * (A @ q) + x[b]
Distribution: pure data-parallel over batch B=16 across 8 NeuronCores
(2 batches per core); gamma replicated. No collectives.

Per-core kernel strategy (PE wall time is stream-cycles + ~45ns/inst, so
everything aims at 1-cycle/row streams and fewer, longer instructions):
  1. All 16 load DMAs (both batches) queued up front on one ring; batch
     0's first chunk is split so the first transpose starts early.
  2. q natural-layout [128, 4, 4096] exact fp32 (residual needs the
     bits); per-chunk fp16 casts qh feed the PE transposes (fp16 streams
     1 cyc/row vs fp32's two-pass LOW_HIGH mode, and keeps the same
     11-bit mantissa as float32r so the suppression argmin stays right).
  3. qT built 2 chunks per PSUM bank (fp16 halves the footprint), one
     PSUM->SBUF copy per pair; Gram matmul in fp16 with the tight
     block-triangle [512,384,256,128] (no f32r >=256 width rule).
  4. E accumulated fp32 in PSUM; strictly-lower blocks mirrored with
     exact fp32 PE transposes (E feeds exp directly).
  5. S = exp(rowmin - E) on ScalarE (bias=rowmin, scale=-1) with
     accum_out Z; S fp16. Softmax is a per-tile pipeline with almost
     nothing else contending for ACT/DVE.
  6. S transposed blockwise on PE (fp16, 1 cyc/row), packed to one
     fp8_e4m3 stationary STdr [128, jt, i] for DoubleRow.
  7. attention-apply in fp8 DoubleRow perf mode: 2 k-tiles per
     instruction halves the instruction count vs fp16. Moving operand =
     fp8 cast of q: batch 0 casts ride a lookahead ring on ScalarE;
     batch 1's are pre-cast on VectorE during its Gram phase so the
     kernel tail never waits on a cast.
  8. Epilogue out = (gamma/Z)*U + x reads exact fp32 q. Injected groups
     put one residual add on PoolE (slow but idle there); tail groups
     stay on VectorE/ScalarE.
  9. Stores batched one [128, 4, 512] DMA per s-chunk (two halves in the
     kernel tail so each fires as soon as 2 epilogue tiles are done); the
     3-deep output ring keeps tail epilogues from stalling on stores.
 10. Cross-batch pipelining: 6 of batch b-1's 8 attention-apply groups
     interleave into batch b's Gram phase; group 6's matmuls fill batch
     b's softmax PE bubble with the epilogue deferred past the softmax
     chain; group 7 runs right after the fp8 stationary is packed.
"""

import sys

import numpy as np

if "/opt/trn_rl_repo" not in sys.path:
    sys.path.insert(0, "/opt/trn_rl_repo")

B, C, H, W = 16, 512, 64, 64
N = H * W                # 4096 spatial positions
P = 128                  # partitions
CT = C // P              # 4 channel tiles
KT = N // P              # 32 contraction chunks for the Gram matmul
FD = 512                 # matmul moving free dim / PSUM bank width (fp32)
NCH = N // FD            # 8 output column chunks
N_CORES = 8
BPC = B // N_CORES       # 2 batches per core

# Moving-operand start column for the upper-triangular Gram matmul (fp16
# streams 1 cyc/row at any width, so the triangle is exact).
MVSTART = [0, 128, 256, 384]

_CACHE = {}


def _build_nc():
    from contextlib import ExitStack

    import concourse.bacc as bacc
    import concourse.tile as tile
    from concourse import mybir
    from concourse.masks import make_identity

    f32 = mybir.dt.float32
    f16 = mybir.dt.float16
    f8e4 = mybir.dt.float8e4
    AF = mybir.ActivationFunctionType
    ALU = mybir.AluOpType
    DR = mybir.MatmulPerfMode.DoubleRow

    nc = bacc.Bacc(None, target_bir_lowering=False)
    # x stays float32 end-to-end on the load path (the DMA cast unit would
    # round); reduced-precision PE operands come from engine casts.
    x_d = nc.dram_tensor("x", [BPC, C, N], f32, kind="ExternalInput")
    g_d = nc.dram_tensor("gamma", [1], f32, kind="ExternalInput")
    o_d = nc.dram_tensor("out", [BPC, C, N], f32, kind="ExternalOutput")

    with ExitStack() as ctx:
        tc = ctx.enter_context(tile.TileContext(nc))
        singles = ctx.enter_context(tc.tile_pool(name="singles", bufs=1))
        bigq = ctx.enter_context(tc.tile_pool(name="bigq", bufs=2))
        qhp = ctx.enter_context(tc.tile_pool(name="qhp", bufs=2))
        qtp = ctx.enter_context(tc.tile_pool(name="qtp", bufs=3))
        qmp = ctx.enter_context(tc.tile_pool(name="qmp", bufs=2))
        mats = ctx.enter_context(tc.tile_pool(name="mats", bufs=4))
        outp = ctx.enter_context(tc.tile_pool(name="outp", bufs=3))
        smallp = ctx.enter_context(tc.tile_pool(name="small", bufs=8))
        psp = ctx.enter_context(tc.tile_pool(name="ps", bufs=8, space="PSUM"))

        def ps_tile(name="ps"):
            return psp.tile([P, FD], f32, tag="ps", name=name)

        def emit_load(b, split_first=False):
            xb = x_d[b].rearrange("(ct p) n -> p ct n", p=P)
            ob = o_d[b].rearrange("(ct p) n -> p ct n", p=P)
            q = bigq.tile([P, CT, N], f32, tag="q")
            for s in range(NCH):
                if split_first and s == 0:
                    h = FD // 2
                    nc.sync.dma_start(out=q[:, :, 0:h], in_=xb[:, :, 0:h])
                    nc.sync.dma_start(out=q[:, :, h:FD], in_=xb[:, :, h:FD])
                    continue
                nc.sync.dma_start(
                    out=q[:, :, s * FD : (s + 1) * FD],
                    in_=xb[:, :, s * FD : (s + 1) * FD],
                )
            return {"q": q, "xb": xb, "ob": ob}

        def emit_qh(st, c, engine="s", halves=False):
            # fp16 cast of q chunk c (feeds the PE transposes)
            q = st["q"]
            qh = qhp.tile([P, CT, FD], f16, tag="qh", name="qh")
            if halves:
                # first transposes only need the first half: don't wait for
                # the whole chunk to land
                h = FD // 2
                nc.scalar.copy(qh[:, :, 0:h], q[:, :, c * FD : c * FD + h])
                nc.scalar.copy(qh[:, :, h:FD], q[:, :, c * FD + h : (c + 1) * FD])
            elif engine == "v":
                nc.vector.tensor_copy(qh[:], q[:, :, c * FD : (c + 1) * FD])
            else:
                nc.scalar.copy(qh[:], q[:, :, c * FD : (c + 1) * FD])
            st["qh"][c] = qh

        def emit_tr_pair(st, kk):
            # transpose chunks 2kk and 2kk+1 into one fp16 PSUM bank;
            # a single PSUM->SBUF copy yields both Gram operand chunks
            pst = psp.tile([P, 2, FD], f16, tag="ps", name="pstr")
            for i in range(2):
                k = 2 * kk + i
                qh = st["qh"][k // 4]
                for t in range(CT):
                    nc.tensor.transpose(
                        pst[:, i, t * P : (t + 1) * P],
                        qh[:, t, (k % 4) * P : (k % 4 + 1) * P],
                        identh[:],
                    )
            qk = qtp.tile([P, 2, C], f16, tag="qt")
            if kk % 2 == 0:
                nc.scalar.copy(qk[:], pst[:])
            else:
                nc.vector.tensor_copy(qk[:], pst[:])
            st["qt"][kk] = qk

        def emit_mm1(st, k):
            qkr = st["qt"][k // 2]
            psE = st["psE"]
            for t in range(CT):
                w = C - MVSTART[t]
                nc.tensor.matmul(
                    psE[t][:, :w],
                    qkr[:, k % 2, t * P : (t + 1) * P],
                    qkr[:, k % 2, MVSTART[t] :],
                    start=(k == 0),
                    stop=(k == KT - 1),
                )

        def emit_cast(st, s, engine):
            # fp8 cast of a q chunk: DoubleRow moving operand for mm2
            q = st["q"]
            qm = qmp.tile(
                [P, CT, FD], f8e4, tag=st["qm_tag"], bufs=st["qm_bufs"], name="qm"
            )
            src = q[:, :, s * FD : (s + 1) * FD]
            if engine == "v":
                nc.vector.tensor_copy(qm[:], src)
            else:
                nc.scalar.copy(qm[:], src)
            st["qms"][s] = qm

        def emit_mm2_t(st, s, t):
            # DoubleRow matmul pair for one (t, s) output tile
            qm = st["qms"][s]
            STdr = st["ST"]
            pu = ps_tile("pu")
            nc.tensor.matmul(
                pu[:],
                STdr[:, 0:2, t * P : (t + 1) * P],
                qm[:, 0:2, :],
                start=True,
                stop=False,
                perf_mode=DR,
            )
            nc.tensor.matmul(
                pu[:],
                STdr[:, 2:4, t * P : (t + 1) * P],
                qm[:, 2:4, :],
                start=False,
                stop=True,
                perf_mode=DR,
            )
            return pu

        def emit_epi_t(st, s, t, pu, ot, pool_add):
            # out = (U * gamma/Z) + x for one (t, s) tile
            q, grz = st["q"], st["grz"]
            xs = q[:, t, s * FD : (s + 1) * FD]
            if t % 2 == 0:
                nc.vector.scalar_tensor_tensor(
                    ot[:, t, :], pu[:], grz[t][:], xs, op0=ALU.mult, op1=ALU.add
                )
            else:
                nc.scalar.mul(ot[:, t, :], pu[:], grz[t][:])
                # pool_add: 2 = both odd tiles on PoolE (PE/ACT/DVE-dense
                # Gram window), 1 = only t=1 (store-paced tail: PoolE and
                # DVE split the adds), 0 = none
                if pool_add >= 2 or (pool_add == 1 and t == 1):
                    nc.gpsimd.tensor_add(ot[:, t, :], ot[:, t, :], xs)
                else:
                    nc.vector.tensor_add(ot[:, t, :], ot[:, t, :], xs)

        def emit_store(st, s, ot):
            nc.sync.dma_start(
                out=st["ob"][:, :, s * FD : (s + 1) * FD], in_=ot[:]
            )

        def emit_mm2_s(st, s, pool_add, split_store=False, t_order=(0, 1, 2, 3)):
            # one full s-chunk: per-t matmul pair + epilogue (<=2 pu live)
            if st["qm_tag"] == "qm0":
                for c in (s, s + 1, s + 2):
                    if c < NCH and st["qms"][c] is None:
                        emit_cast(st, c, "s")
            ot = outp.tile([P, CT, FD], f32, tag="ot")
            done = set()
            for t in t_order:
                pu = emit_mm2_t(st, s, t)
                emit_epi_t(st, s, t, pu, ot, pool_add)
                done.add(t)
                # fire each half-store as soon as its two tiles are done
                if split_store and done >= {0, 1} and "lo" not in done:
                    done.add("lo")
                    nc.sync.dma_start(
                        out=st["ob"][:, 0:2, s * FD : (s + 1) * FD],
                        in_=ot[:, 0:2, :],
                    )
                if split_store and done >= {2, 3} and "hi" not in done:
                    done.add("hi")
                    nc.sync.dma_start(
                        out=st["ob"][:, 2:4, s * FD : (s + 1) * FD],
                        in_=ot[:, 2:4, :],
                    )
            if not split_store:
                emit_store(st, s, ot)

        def emit_gram(st, prev, skip_chunks=0, precast=False):
            """Transposes + Gram matmul for `st`, burst-interleaved with the
            previous batch's attention-apply (mm2) so PE never idles long
            enough for the HAM clock gate to re-throttle. Batch `st`'s own
            fp8 moving-operand casts are pre-issued on VectorE here."""
            st["psE"] = [ps_tile("psE") for _ in range(CT)]
            if "qt" not in st:
                st["qt"] = [None] * (KT // 2)
            if "qh" not in st:
                st["qh"] = [None] * NCH
                emit_qh(st, 0, halves=True)
            LOOKC = 2  # chunks of pipeline distance between tr and mm1
            for kk in range(skip_chunks // 2, KT // 2):
                # fp16 cast one load-chunk ahead of the transposes
                if kk % 2 == 0:
                    for c in (kk // 2, kk // 2 + 1):
                        if c < NCH and st["qh"][c] is None:
                            emit_qh(st, c, "s" if kk % 4 else "v")
                emit_tr_pair(st, kk)
                for k in (2 * kk - LOOKC, 2 * kk + 1 - LOOKC):
                    if k >= 0:
                        emit_mm1(st, k)
                # pre-cast this batch's fp8 moving chunks (consumed by its
                # own mm2 in the kernel tail, where a cast stall would be
                # serial)
                if precast and kk % 2 == 1 and st["qms"][kk // 2] is None:
                    emit_cast(st, kk // 2, "v")
                # 6 of 8 s-groups of the previous batch's attention-apply;
                # the last two fill this batch's own softmax phase
                if prev is not None and kk % 2 == 1 and kk // 2 < NCH - 2:
                    emit_mm2_s(prev, kk // 2, pool_add=1)
            for k in (KT - 2, KT - 1):
                emit_mm1(st, k)

        def emit_softmax(st, prev=None):
            # ---- copy E out of PSUM; mirror strictly-lower blocks ----
            psE = st["psE"]
            E = []
            for t in range(CT):
                e = mats.tile([P, FD], f32, tag="E")
                w = C - MVSTART[t]
                if t % 2 == 0:
                    nc.scalar.copy(e[:, MVSTART[t] :], psE[t][:, :w])
                else:
                    nc.vector.tensor_copy(e[:, MVSTART[t] :], psE[t][:, :w])
                E.append(e)
            # row-tile 0 needs no mirrors: its rowmin goes first so exp0
            # starts while the mirrors are still being copied
            rms = [smallp.tile([P, 1], f32, tag="rm", name="rm") for _ in range(CT)]
            nc.vector.tensor_reduce(
                rms[0][:], E[0][:], axis=mybir.AxisListType.X, op=ALU.min
            )
            # E[t][:, s-block] = E[s][:, t-block].T for s < t (exact fp32
            # transposes: E magnitudes are ~4e3 and feed exp directly, so
            # low-precision rounding here would be a real error). Tile 3's
            # mirrors go first to match the exp emission order below.
            for t in (3, 2, 1):
                for s in range(t):
                    pm = ps_tile("pm")
                    nc.tensor.transpose(
                        pm[:, :P], E[s][:, t * P : (t + 1) * P], ident[:]
                    )
                    if (t + s) % 2 == 0:
                        nc.scalar.copy(E[t][:, s * P : (s + 1) * P], pm[:, :P])
                    else:
                        nc.vector.tensor_copy(
                            E[t][:, s * P : (s + 1) * P], pm[:, :P]
                        )

            # PE bubble fill: the previous batch's s-group 6 runs as bare
            # matmuls (epilogue deferred past the softmax chain so ACT/DVE
            # stay clear); for the first batch, the NEXT batch's first
            # transposes fill in instead.
            pus6 = None
            if prev is not None:
                pus6 = [emit_mm2_t(prev, NCH - 2, t) for t in range(CT)]
            elif st.get("next") is not None:
                nxt = st["next"]
                nxt["qh"] = [None] * NCH
                nxt["qt"] = [None] * (KT // 2)
                emit_qh(nxt, 0, "s")
                emit_tr_pair(nxt, 0)
                emit_qh(nxt, 1, "v")
                # this batch's first fp8 chunks: cast now so the first
                # injected attention-apply group doesn't stall on them
                emit_cast(st, 0, "s")
                emit_cast(st, 1, "v")

            # ---- suppression softmax: S = exp(rowmin - E), Z = rowsum(S),
            # pipelined per row-tile with the S transposes ----
            pstS = [
                psp.tile([P, FD], f16, tag="ps", name="pstS") for _ in range(CT)
            ]
            grz = [None] * CT
            for t in (0, 3, 2, 1):  # match mirror readiness order
                if t > 0:
                    nc.vector.tensor_reduce(
                        rms[t][:], E[t][:], axis=mybir.AxisListType.X, op=ALU.min
                    )
                s_t = mats.tile([P, FD], f16, tag="S")
                z = smallp.tile([P, 1], f32, tag="z")
                nc.scalar.activation(
                    s_t[:], E[t][:], AF.Exp, bias=rms[t][:], scale=-1.0,
                    accum_out=z[:],
                )
                for jt in range(CT):
                    nc.tensor.transpose(
                        pstS[jt][:, t * P : (t + 1) * P],
                        s_t[:, jt * P : (jt + 1) * P],
                        identh[:],
                    )
                rz = smallp.tile([P, 1], f32, tag="rz")
                nc.vector.reciprocal(rz[:], z[:])
                g = smallp.tile([P, 1], f32, tag="grz")
                nc.gpsimd.tensor_mul(g[:], rz[:], gam[:])
                grz[t] = g
            st["grz"] = grz

            # ---- STdr = S.T packed fp8_e4m3 (DoubleRow stationary) ----
            STdr = mats.tile([P, CT, C], f8e4, tag="ST", bufs=2)
            for jt in range(CT):
                if jt % 2 == 0:
                    nc.scalar.copy(STdr[:, jt, :], pstS[jt][:])
                else:
                    nc.vector.tensor_copy(STdr[:, jt, :], pstS[jt][:])
            st["ST"] = STdr

            # deferred epilogue of the bubble-fill group, then the final
            # s-group of the previous batch
            if prev is not None:
                ot6 = outp.tile([P, CT, FD], f32, tag="ot", name="ot6")
                for t in range(CT):
                    emit_epi_t(prev, NCH - 2, t, pus6[t], ot6, pool_add=1)
                emit_store(prev, NCH - 2, ot6)
                emit_mm2_s(prev, NCH - 1, pool_add=1, split_store=True)

        # ---- pipelined driver: batch b's Gram phase overlaps batch b-1's
        # attention-apply phase on the PE ----
        st0 = emit_load(0, split_first=True)
        st1 = emit_load(1)

        ident = singles.tile([P, P], f32)
        make_identity(nc, ident)
        identh = singles.tile([P, P], f16)
        nc.gpsimd.tensor_copy(identh[:], ident[:])

        # gamma broadcast to all partitions as a per-partition scalar
        gam = singles.tile([P, 1], f32)
        nc.gpsimd.dma_start(out=gam[:], in_=g_d[:].to_broadcast([P, 1]))

        # batch 0's fp8 casts ride a ScalarE lookahead ring (its mm2 is
        # interleaved into batch 1's Gram phase, so a ring is fine there);
        # batch 1's are all pre-cast during its Gram phase.
        st0["qm_tag"], st0["qm_bufs"], st0["qms"] = "qm0", 3, [None] * NCH
        st1["qm_tag"], st1["qm_bufs"], st1["qms"] = "qm1", NCH, [None] * NCH
        emit_gram(st0, None)
        st0["next"] = st1
        emit_softmax(st0, None)
        emit_gram(st1, st0, skip_chunks=2, precast=True)
        emit_softmax(st1, st0)
        for s in range(NCH):
            emit_mm2_s(st1, s, pool_add=0, split_store=True)

    nc.compile()
    return nc


def _get_nc():
    if "nc" not in _CACHE:
        _CACHE["nc"] = _build_nc()
    return _CACHE["nc"]


def kernel(x: np.ndarray, gamma: np.ndarray) -> np.ndarray:
    from concourse.bass_utils import run_bass_kernel_spmd

    nc = _get_nc()
    x = np.ascontiguousarray(np.asarray(x, dtype=np.float32))
    gamma = np.ascontiguousarray(np.asarray(gamma, dtype=np.float32))
    xs = x.reshape(B, C, N)
    in_maps = [
        {
            "x": np.ascontiguousarray(xs[c * BPC : (c + 1) * BPC]),
            "gamma": gamma,
        }
        for c in range(N_CORES)
    ]
    res = run_bass_kernel_spmd(nc, in_maps, core_ids=list(range(N_CORES)))
    out = np.stack([res.results[c]["out"] for c in range(N_CORES)], axis=0)
    return out.reshape(B, C, H, W)
